# revision 51
# baseline (speedup 1.0000x reference)
"""Trainium2 Bass kernel for nn_MultiHeadAttention_9405978378694.

Full-input contract: kernel(**inputs) -> (B, S, DM) float32.

Sharding: tensor-parallel over heads. 16 heads / 8 cores = 2 heads per core.
Each core computes QKV projection for its heads (full sequence, both
batches), causal attention, and a partial out-projection against its slice
of w_out columns. Partials are summed on the host (the all-reduce).

Host-side algebra (exact):
  - The reference applies rotary with frequencies indexed by the HEAD axis
    (not position), so each head's rotation is a constant 128x128 linear map.
    It is folded into the Q/K projection weights: Wq' = R_h @ Wq_h.
  - The V bias enters the output as attn_rows_sum(=1) * b_v, which passes
    linearly through the out-projection: folded into the final bias add as
    w_out @ b_v.

Device kernel (per core, all matmuls bf16 with fp32 PSUM accumulation):
  QKV in transposed [dh, s] layout -> scores computed TRANSPOSED
  (lhsT = k block, rhs = 256 q cols = two query tiles per stream, so each
  k block's LDWEIGHTS hides under the previous matmul) giving attn [t, s]
  tiles already in the layout attn@V needs as its stationary operand — no
  PE transposes of the attention matrix at all -> exp on ScalarE ->
  attn@V with lhsT = attnT block, rhs = V augmented with a ones column
  ([t, d | 1]), so the softmax row-sum pops out of the same accumulation
  as column 128 of the output PSUM for free -> per-partition 1/rowsum
  normalization at the o eviction (DVE tensor_scalar with the VectorE
  reciprocal of the rowsum) -> one small PE transpose of o per (si, head)
  for the out-projection -> out-projection with both heads accumulated in
  one PSUM group; output written bf16 (partials summed on host in f64).

Scheduling notes (all measured on HW; ~334us/core, PE ~94% busy):
  - software pipeline per tile: otransp(si-1), attnv(si), outproj(si-1),
    with scores/exp emitted two PAIRS ahead (pairs 0/1 during the QKV
    chunks, pair p+2 at si=2p+1), so neither the softmax chain nor the
    normalization chain ever gates the PE; 512-col score streams (quads)
    measurably REGRESS — they outrun the ScalarE exp 2.5x and stall the
    3-bank PSUM ring;
  - engine balance: ScalarE owns exp + qkv/oT evictions, VectorE owns
    causal-bias adds, reciprocal, normalize and ostage evictions, GpSimd
    owns nothing compute (it cannot touch PSUM) but issues the bulk xt
    loads on the SWDGE rings so the HWDGE FIFOs (sync=stores,
    scalar=startup weights) never block output tiles behind loads;
  - startup DMA: per-k-tile weights host-concatenated into [16, 128, 768]
    (1.5KB contiguous lines instead of 256B descriptors), interleaved
    kt-major with the xt first-chunk pieces across sync/scalar/gpsimd
    rings; the b0 first-chunk QKV runs kt-outer consuming tiles as they
    land; a short identity-matmul warmup releases the PE HAM clock gate
    (ident + causal bias + biases all arrive via leading DMAs).
"""

import os
import numpy as np
import ml_dtypes

B, S, DM, H, DH = 2, 2048, 2048, 16, 128
NCORES = 8
HPC = H // NCORES  # heads per core
NT = S // 128      # 128-row tiles along sequence
SCALE = float(DH) ** -0.5
WCAT = 4 * DH + HPC * DH  # 768: wq0|wk0|wq1|wk1|wv(h0|h1) per k-tile
VROW = HPC * (DH + 1)     # 258: v row block: h0 d0..127,1 | h1 d0..127,1

_BUILT = {}
_LAST_IN_MAPS = None


def _build(causal: bool):
    import concourse.mybir as mybir
    import concourse.tile as tile
    from concourse import bacc

    f32 = mybir.dt.float32
    bf16 = mybir.dt.bfloat16
    AF = mybir.ActivationFunctionType

    nc = bacc.Bacc("TRN2", target_bir_lowering=False, debug=False)

    xt = nc.dram_tensor("xt", [B, DM, S], bf16, kind="ExternalInput")
    wcat = nc.dram_tensor("wcat", [16, 128, WCAT], bf16, kind="ExternalInput")
    identd = nc.dram_tensor("identb", [128, 128], bf16, kind="ExternalInput")
    cbiasd = nc.dram_tensor("cbias", [128, 128], f32, kind="ExternalInput")
    bqk = nc.dram_tensor("bqk", [128, 2 * HPC], f32, kind="ExternalInput")
    wo = nc.dram_tensor("wo", [HPC, DH, DM], bf16, kind="ExternalInput")
    if not causal:
        mb = nc.dram_tensor("maskbT", [S, S], f32, kind="ExternalInput")
    outp = nc.dram_tensor("outp", [B, S, DM], bf16, kind="ExternalOutput")

    from contextlib import ExitStack
    with tile.TileContext(nc) as tc:
        with ExitStack() as es:
            constp = es.enter_context(tc.tile_pool(name="const", bufs=1))
            wp = es.enter_context(tc.tile_pool(name="wp", bufs=1))
            wop = es.enter_context(tc.tile_pool(name="wop", bufs=1))
            xtp = es.enter_context(tc.tile_pool(name="xtp", bufs=1))
            qkvp = es.enter_context(tc.tile_pool(name="qkv", bufs=1))
            attnTp = es.enter_context(tc.tile_pool(name="attnT", bufs=2))
            accp = es.enter_context(tc.tile_pool(name="accs", bufs=2))
            osbp = es.enter_context(tc.tile_pool(name="osb", bufs=2))
            oTsbp = es.enter_context(tc.tile_pool(name="oTsb", bufs=2))
            outsp = es.enter_context(tc.tile_pool(name="ostage", bufs=3))
            mbp = es.enter_context(tc.tile_pool(name="mbp", bufs=3))
            Ap = es.enter_context(tc.tile_pool(name="A", bufs=3, space="PSUM"))
            psop = es.enter_context(tc.tile_pool(name="pso", bufs=2, space="PSUM"))
            pop = es.enter_context(tc.tile_pool(name="po", bufs=1, space="PSUM"))
            oTp = es.enter_context(tc.tile_pool(name="oT", bufs=1, space="PSUM"))

            # identity + causal bias come in via the first DMAs (cheaper
            # startup latency than building them on gpsimd)
            ident = constp.tile([128, 128], bf16)
            nc.sync.dma_start(out=ident[:], in_=identd.ap()[:])
            cbias = constp.tile([128, 128], f32)

            # one big weight tile; per-kt slices at kt*WCAT + off
            wcat_sb = wp.tile([128, 16 * WCAT], bf16, name="wcat_sb")

            def wq_ap(h, kt):
                return wcat_sb[:, kt * WCAT + 2 * h * DH:
                               kt * WCAT + (2 * h + 1) * DH]

            def wk_ap(h, kt):
                return wcat_sb[:, kt * WCAT + (2 * h + 1) * DH:
                               kt * WCAT + (2 * h + 2) * DH]

            def wv_ap(kt):
                return wcat_sb[:, kt * WCAT + 4 * DH:(kt + 1) * WCAT]

            wo_t = [wop.tile([128, DM], bf16, tag=f"wo{h}", name=f"wo{h}")
                    for h in range(HPC)]
            # one [128, 4] tile: cols = bq0, bk0, bq1, bk1
            bqk_t = constp.tile([128, 2 * HPC], f32, name="bqk_t")
            bq_t = [bqk_t[:, 2 * h:2 * h + 1] for h in range(HPC)]
            bk_t = [bqk_t[:, 2 * h + 1:2 * h + 2] for h in range(HPC)]

            # warmup: release the PE HAM clock gate while startup DMA lands
            warm_ps = psop.tile([128, 512], f32, tag="pso", name="warm_ps")
            for _ in range(36):
                nc.tensor.matmul(warm_ps[:, :128], lhsT=ident[:], rhs=ident[:],
                                 start=True, stop=True)
            nc.scalar.dma_start(out=bqk_t[:], in_=bqk.ap()[:])

            xts = [xtp.tile([128, S], bf16, tag=f"xt{kt}", name=f"xt_{kt}")
                   for kt in range(16)]
            # kt-major interleave: each kt's weights land just before its
            # xt piece, striped across both HWDGE queues + the SWDGE rings
            for kt in range(16):
                eng = (nc.scalar, nc.gpsimd, nc.sync)[kt % 3]
                eng.dma_start(out=wcat_sb[:, kt * WCAT:(kt + 1) * WCAT],
                              in_=wcat.ap()[kt])
                eng.dma_start(out=xts[kt][:, 0:512],
                              in_=xt.ap()[0, kt * 128:(kt + 1) * 128, 0:512])
            nc.scalar.dma_start(out=cbias[:], in_=cbiasd.ap()[:])
            # remaining b0 s-cols: one wide (3KB-line) transfer per kt,
            # striped across all three rings (each ring's startup pieces
            # drain first, then its rests follow) so the last rest lands
            # ~20us in instead of ~36us on a single SWDGE FIFO
            for kt in range(16):
                eng = (nc.gpsimd, nc.sync, nc.scalar)[kt % 3]
                eng.dma_start(
                    out=xts[kt][:, 512:2048],
                    in_=xt.ap()[0, kt * 128:(kt + 1) * 128, 512:2048])
            for h in range(HPC):
                nc.sync.dma_start(out=wo_t[h][:], in_=wo.ap()[h])

            for b in range(B):
                if b > 0:
                    # SWDGE (gpsimd) rings: keeps the HWDGE FIFOs free for
                    # the output-tile stores issued during b0's attention
                    for kt in range(16):
                        nc.gpsimd.dma_start(
                            out=xts[kt][:],
                            in_=xt.ap()[b, kt * 128:(kt + 1) * 128, :])

                q_sb = [qkvp.tile([128, S], bf16, tag=f"q{h}", name=f"q_{b}_{h}")
                        for h in range(HPC)]
                k_sb = [qkvp.tile([128, S], bf16, tag=f"k{h}", name=f"k_{b}_{h}")
                        for h in range(HPC)]
                # V both heads + ones cols: [t_local, st*258 + h*129 + d]
                v_sb = qkvp.tile([128, 16 * VROW], bf16, tag="v", name=f"v_{b}")
                # ones lanes (cols st*258 + h*129 + 128)
                nc.gpsimd.memset(
                    v_sb[:].rearrange("p (st h x) -> p st h x",
                                      st=16, h=HPC)[:, :, :, DH:DH + 1], 1.0)

                # ---- attention (scores computed transposed) ----
                def stage_scores_pair(pi):
                    """scoresT + exp for query tiles si0=2pi, si1=2pi+1, both
                    heads. 256-col streams (both s-tiles at once) so the
                    LDWEIGHTS of each k block hides under the previous MM.
                    aT col region j = [t(128), s(si0)|s(si1)] (256 cols).
                    Block (j=si1, si0-half) is junk (fully-causal-masked) —
                    attnv(si0) simply never reads it."""
                    si0 = 2 * pi
                    nj = si0 + 2 if causal else NT
                    attnT_h = []
                    for h in range(HPC):
                        aT = attnTp.tile([128, nj * 256], bf16, tag=f"aT{h}",
                                         name=f"aT_{b}_{pi}_{h}")
                        for c in range(0, nj, 2):
                            jn = min(2, nj - c)
                            ps = Ap.tile([128, 512], f32, tag="A",
                                         name=f"ps_{b}_{pi}_{h}_{c}")
                            for j2 in range(jn):
                                j = c + j2
                                nc.tensor.matmul(
                                    ps[:, j2 * 256:(j2 + 1) * 256],
                                    lhsT=k_sb[h][:, j * 128:(j + 1) * 128],
                                    rhs=q_sb[h][:, si0 * 128:si0 * 128 + 256],
                                    start=True, stop=True)
                            if causal and c + jn == nj:
                                # diag of si0 (block j=si0, first half) and
                                # diag of si1 (block j=si0+1, second half)
                                nc.vector.tensor_add(
                                    ps[:, 0:128], ps[:, 0:128], cbias[:])
                                nc.vector.tensor_add(
                                    ps[:, 384:512], ps[:, 384:512], cbias[:])
                            elif not causal:
                                for j2 in range(jn):
                                    j = c + j2
                                    mt = mbp.tile([128, 256], f32, tag="mb",
                                                  name=f"mb_{b}_{pi}_{h}_{c}_{j2}")
                                    nc.sync.dma_start(
                                        out=mt[:],
                                        in_=mb.ap()[j * 128:(j + 1) * 128,
                                                    si0 * 128:si0 * 128 + 256])
                                    nc.vector.tensor_add(
                                        ps[:, j2 * 256:(j2 + 1) * 256],
                                        ps[:, j2 * 256:(j2 + 1) * 256], mt[:])
                            nc.scalar.activation(
                                aT[:, c * 256:(c + jn) * 256],
                                ps[:, :jn * 256], AF.Exp, scale=SCALE)
                        attnT_h.append(aT)
                    return attnT_h

                def stage_attnv(si, attnT_h):
                    """attn@V with ones-augmented V; normalize at eviction."""
                    nj = si + 1 if causal else NT
                    half = (si % 2) * 128
                    o_h = []
                    for h in range(HPC):
                        po = pop.tile([128, 512], f32, tag=f"po{h}",
                                      name=f"po_{b}_{si}_{h}")
                        for j in range(nj):
                            nc.tensor.matmul(
                                po[:, :DH + 1],
                                lhsT=attnT_h[h][:, j * 256 + half:
                                                j * 256 + half + 128],
                                rhs=v_sb[:, j * VROW + h * (DH + 1):
                                         j * VROW + (h + 1) * (DH + 1)],
                                start=(j == 0), stop=(j == nj - 1))
                        rinv = accp.tile([128, 1], f32, tag=f"rinv{h}",
                                         name=f"rinv_{b}_{si}_{h}")
                        nc.vector.reciprocal(rinv[:], po[:, DH:DH + 1])
                        o_sb = osbp.tile([128, DH], bf16, tag=f"o{h}",
                                         name=f"o_{b}_{si}_{h}")
                        nc.vector.tensor_scalar_mul(o_sb[:], po[:, :DH],
                                                    rinv[:])
                        o_h.append(o_sb)
                    return o_h

                def stage_otransp(si, o_h):
                    """PE transpose of o + DVE eviction (emitted early so the
                    eviction overlaps the next tile's scores matmuls)."""
                    oTps = oTp.tile([128, HPC * DH], f32, tag="oT",
                                    name=f"oTps_{b}_{si}")
                    for h in range(HPC):
                        nc.tensor.matmul(
                            oTps[:, h * DH:(h + 1) * DH],
                            lhsT=o_h[h][:], rhs=ident[:],
                            start=True, stop=True)
                    oT = oTsbp.tile([128, HPC * DH], bf16, tag="oTsb",
                                    name=f"oT_{b}_{si}")
                    nc.vector.tensor_copy(oT[:], oTps[:])
                    return oT

                def stage_out(si, oT):
                    """out-projection -> staged DMA out."""
                    ostage = outsp.tile([128, DM], bf16, tag="ostage",
                                        name=f"ostage_{b}_{si}")
                    for ncn in range(4):
                        nsl = slice(ncn * 512, (ncn + 1) * 512)
                        pso = psop.tile([128, 512], f32, tag="pso",
                                        name=f"pso_{b}_{si}_{ncn}")
                        for h in range(HPC):
                            nc.tensor.matmul(pso[:],
                                             lhsT=oT[:, h * DH:(h + 1) * DH],
                                             rhs=wo_t[h][:, nsl],
                                             start=(h == 0), stop=(h == HPC - 1))
                        if ncn % 2 == 0:
                            nc.vector.tensor_copy(ostage[:, nsl], pso[:])
                        else:
                            nc.scalar.activation(ostage[:, nsl], pso[:],
                                                 AF.Copy)
                        last_tile = (b == B - 1 and si == NT - 1)
                        if last_tile:
                            nc.sync.dma_start(
                                out=outp.ap()[b, si * 128:(si + 1) * 128, nsl],
                                in_=ostage[:, nsl])
                        elif ncn % 2 == 1:
                            nc.sync.dma_start(
                                out=outp.ap()[b, si * 128:(si + 1) * 128,
                                              (ncn - 1) * 512:(ncn + 1) * 512],
                                in_=ostage[:, (ncn - 1) * 512:(ncn + 1) * 512])

                # ---- QKV projection ----
                if b == 0:
                    # kt-outer for the first s-chunk: consume each piece as
                    # it arrives from HBM instead of waiting for all 16
                    ssl = slice(0, 512)
                    psqk = [[(Ap.tile([128, 512], f32, tag="A",
                                      name=f"psqk0_{h}_{qk}")
                              if (h, qk) != (HPC - 1, 1) else
                              psop.tile([128, 512], f32, tag="pso",
                                        name=f"psqk0_{h}_{qk}"))[:]
                             for qk in range(2)] for h in range(HPC)]
                    psv4 = [pop.tile([128, 512], f32, tag=f"po{st}",
                                     name=f"psv0_{st}") if st < 2 else
                            (oTp.tile([128, 256], f32, tag="oT",
                                      name=f"psv0_{st}") if st == 2 else
                             psop.tile([128, 512], f32, tag="pso",
                                       name=f"psv0_{st}"))
                            for st in range(4)]
                    for kt in range(16):
                        for h in range(HPC):
                            nc.tensor.matmul(
                                psqk[h][0], lhsT=wq_ap(h, kt),
                                rhs=xts[kt][:, ssl],
                                start=(kt == 0), stop=(kt == 15))
                            nc.tensor.matmul(
                                psqk[h][1], lhsT=wk_ap(h, kt),
                                rhs=xts[kt][:, ssl],
                                start=(kt == 0), stop=(kt == 15))
                        for st in range(4):
                            nc.tensor.matmul(
                                psv4[st][:, :HPC * DH],
                                lhsT=xts[kt][:, st * 128:(st + 1) * 128],
                                rhs=wv_ap(kt),
                                start=(kt == 0), stop=(kt == 15))
                    for h in range(HPC):
                        nc.scalar.activation(q_sb[h][:, ssl], psqk[h][0],
                                             AF.Identity, bias=bq_t[h])
                        nc.scalar.activation(k_sb[h][:, ssl], psqk[h][1],
                                             AF.Identity, bias=bk_t[h])
                    for st in range(4):
                        for h in range(HPC):
                            nc.scalar.activation(
                                v_sb[:, st * VROW + h * (DH + 1):
                                     st * VROW + h * (DH + 1) + DH],
                                psv4[st][:, h * DH:(h + 1) * DH], AF.Copy)
                pre_pairs = []
                for sc4 in range(4):
                    if sc4 in (2, 3) and len(pre_pairs) < sc4 - 1:
                        # pairs 0 and 1 only need the sc0 q/k columns: their
                        # matmuls fill any PE dep-stalls in the later QKV
                        # chunks and pre-warm the attention pipeline
                        pre_pairs.append(stage_scores_pair(len(pre_pairs)))
                    if b == 0 and sc4 == 0:
                        continue
                    ssl = slice(sc4 * 512, (sc4 + 1) * 512)
                    for h in range(HPC):
                        for wap, bt, dst in ((wq_ap, bq_t, q_sb),
                                             (wk_ap, bk_t, k_sb)):
                            ps = Ap.tile([128, 512], f32, tag="A")
                            for kt in range(16):
                                nc.tensor.matmul(
                                    ps[:], lhsT=wap(h, kt),
                                    rhs=xts[kt][:, ssl],
                                    start=(kt == 0), stop=(kt == 15))
                            nc.scalar.activation(dst[h][:, ssl], ps[:],
                                                 AF.Identity, bias=bt[h])
                    for st4 in range(4):
                        st = sc4 * 4 + st4
                        tsl = slice(st * 128, (st + 1) * 128)
                        psv = psop.tile([128, 512], f32, tag="pso")
                        for kt in range(16):
                            nc.tensor.matmul(
                                psv[:, :HPC * DH], lhsT=xts[kt][:, tsl],
                                rhs=wv_ap(kt),
                                start=(kt == 0), stop=(kt == 15))
                        for h in range(HPC):
                            nc.scalar.activation(
                                v_sb[:, st * VROW + h * (DH + 1):
                                     st * VROW + h * (DH + 1) + DH],
                                psv[:, h * DH:(h + 1) * DH], AF.Copy)

                # pipeline: otransp(si-1), attnv(si), outproj(si-1),
                # scores-pair(si//2+2) on odd si — pairs 0/1 were emitted
                # during the QKV chunks, so scores stay two pairs ahead and
                # the exp chain never gates attnv
                aT_cur, aT_nxt = pre_pairs
                o_prev = None
                for si in range(NT):
                    oT_prev = (stage_otransp(si - 1, o_prev)
                               if o_prev is not None else None)
                    o_cur = stage_attnv(si, aT_cur)
                    if oT_prev is not None:
                        stage_out(si - 1, oT_prev)
                    o_prev = o_cur
                    if si % 2 == 1:
                        aT_cur = aT_nxt
                        if si // 2 + 2 < NT // 2:
                            aT_nxt = stage_scores_pair(si // 2 + 2)
                oT_prev = stage_otransp(NT - 1, o_prev)
                stage_out(NT - 1, oT_prev)

    nc.compile()
    return nc


def _get(causal: bool):
    if causal not in _BUILT:
        _BUILT[causal] = _build(causal)
    return _BUILT[causal]


def _rot(fr, fi, m):
    """Apply the reference's per-head rotary as a linear map on rows of m."""
    top, bot = m[:DH // 2], m[DH // 2:]
    return np.concatenate([fr[:, None] * top - fi[:, None] * bot,
                           fi[:, None] * top + fr[:, None] * bot], axis=0)


def kernel(x, w_qkv, b_qkv, w_out, b_out, fc_real, fc_imag, mask):
    x = np.asarray(x, np.float32)
    w_qkv = np.asarray(w_qkv, np.float32)
    b_qkv = np.asarray(b_qkv, np.float32)
    w_out = np.asarray(w_out, np.float32)
    b_out = np.asarray(b_out, np.float32)
    fc_real = np.asarray(fc_real, np.float32)
    fc_imag = np.asarray(fc_imag, np.float32)
    mask_np = np.asarray(mask)[0, 0]

    causal = bool(np.array_equal(
        mask_np, np.triu(np.ones((S, S), bool), 1)))

    bf = ml_dtypes.bfloat16
    xt_host = np.ascontiguousarray(x.transpose(0, 2, 1)).astype(bf)

    in_maps = []
    maskbT = None
    if not causal:
        maskbT = np.ascontiguousarray(
            np.where(mask_np, np.float32(-1e30), np.float32(0.0)).T)
    for c in range(NCORES):
        bq_h, bk_h, wo_h = [], [], []
        wq_h, wk_h, wv_h = [], [], []
        for hh in range(HPC):
            g = c * HPC + hh
            fr = fc_real[0, g, :]
            fi = fc_imag[0, g, :]
            wq_h.append(np.ascontiguousarray(
                _rot(fr, fi, w_qkv[g * DH:(g + 1) * DH, :]).T))
            wk_h.append(np.ascontiguousarray(
                _rot(fr, fi, w_qkv[DM + g * DH:DM + (g + 1) * DH, :]).T))
            bq_h.append(_rot(fr, fi, b_qkv[g * DH:(g + 1) * DH, None])[:, 0])
            bk_h.append(_rot(fr, fi,
                             b_qkv[DM + g * DH:DM + (g + 1) * DH, None])[:, 0])
            wv_h.append(w_qkv[2 * DM + g * DH:2 * DM + (g + 1) * DH, :].T)
            wo_h.append(np.ascontiguousarray(
                w_out[:, g * DH:(g + 1) * DH].T).astype(bf))
        # per-kt concat: wq0|wk0|wq1|wk1|wv(h0|h1) -> [16, 128, WCAT]
        wall = np.concatenate(
            [wq_h[0], wk_h[0], wq_h[1], wk_h[1], wv_h[0], wv_h[1]],
            axis=1)  # [DM, WCAT]
        wcat = np.ascontiguousarray(
            wall.reshape(16, 128, WCAT)).astype(bf)
        tt, ss = np.meshgrid(np.arange(128), np.arange(128), indexing="ij")
        m = {
            "xt": xt_host,
            "wcat": wcat,
            "identb": np.eye(128, dtype=np.float32).astype(bf),
            "cbias": np.where(ss >= tt, np.float32(0.0),
                              np.float32(-1e30)),
            "bqk": np.stack([bq_h[0], bk_h[0], bq_h[1], bk_h[1]],
                            axis=1).astype(np.float32),
            "wo": np.stack(wo_h),
        }
        if not causal:
            m["maskbT"] = maskbT
        in_maps.append(m)

    from concourse.bass_utils import run_bass_kernel_spmd
    nc = _get(causal)
    global _LAST_IN_MAPS
    _LAST_IN_MAPS = in_maps
    trace = os.environ.get("MHA_TRACE") == "1"
    res = run_bass_kernel_spmd(nc, in_maps, core_ids=list(range(NCORES)),
                               trace=trace)
    if trace:
        kernel.last_results = res

    out = res.results[0]["outp"].astype(np.float64)
    for c in range(1, NCORES):
        out += res.results[c]["outp"].astype(np.float64)
    b_v = b_qkv[2 * DM:]
    out += (b_out + w_out @ b_v)[None, None, :]
    return out.astype(np.float32)
